# revision 4
# baseline (speedup 1.0000x reference)
"""AdjMultiHeadAttention Trainium2 kernel.

Sharding: pure data-parallel over batch. B=16 batches over 8 NeuronCores
-> 2 batches per core. Weights replicated. No collectives.

Per-core algorithm (compute in bf16 on the TensorEngine, fp32 PSUM):
  - host pre-transposes x -> xT [E,S] and mask=(adj+bond) -> maskT [sk,sq],
    pre-transposes weights, folds the 1/sqrt(d) scale into Wq.
  - q/k projections produce qT/kT [f, s]; v natural [s, f].
  - scores are computed transposed: scoresT[sk,sq]; softmax denominator
    comes from a ones-column appended to v accumulated in the ctx matmul.
  - per-head ROUTING spreads the elementwise softmax work across the
    Vector (DVE), Activation (ACT) and GpSimd (Pool) engines:
      * drain 'dve': one DVE scalar_tensor_tensor fuses the PSUM drain
        with the mask multiply (PSUM f32 -> SBUF 16-bit).
      * drain 'act': ACT copies PSUM->SBUF (16-bit), then a DVE
        all-SBUF mask multiply runs in 4x perf mode (0.26 ns/elem/lane).
      * exp 'act': true exp on the Activation engine.
      * exp 'pool'/'dve': Schraudolph fast exp. The drain stage already
        multiplied by C1=128/ln2 (folded into the stt scalar, fp16
        intermediate); the remaining +C2 & round-to-int16 lands on the
        idle GpSimd engine (or DVE in 4x mode). The int16 bits ARE the
        bf16 representation of exp(x) (~1.8% rms per weight; measured
        +2.85e-3 end-to-end error per converted head, in quadrature).
  - ctx normalization is one tensor_tensor per head-PAIR: reciprocal of
    the two denominators, then in1 = rc broadcast along free via a
    stride-0 access pattern (no per-head scalar ops).
  - ctx rows are transposed 128x64-block-wise on the TensorEngine into
    bf16 PSUM; the ctxT drain runs on DVE in 2x_1p mode.

Emission is software-pipelined over units u = (batch, head-pair) with a
backlog-step pump that keeps the TensorEngine (the binding engine after
rebalancing, ~150us busy) fed between elementwise ops.
"""

import sys

sys.path.insert(0, "/opt/trn_rl_repo")

from contextlib import ExitStack

import ml_dtypes
import numpy as np

import concourse.bass as bass
import concourse.tile as tile
from concourse import bacc, mybir
from concourse.bass_utils import run_bass_kernel_spmd
from concourse.masks import make_identity

B, S, E, H, D = 16, 1024, 512, 8, 64
NCORES = 8
BPC = B // NCORES  # batches per core
SCALE = D**-0.5
BF16 = mybir.dt.bfloat16
FP16 = mybir.dt.float16
I16 = mybir.dt.int16
F32 = mybir.dt.float32
NPBF16 = ml_dtypes.bfloat16

_cache = {}

NU = BPC * 4  # pipeline units: (batch, head-pair)

# Schraudolph fast-exp constants: int16(x*C1 + C2) are the bf16 bits of exp(x)
C1 = 184.66496
C2 = 16248.577

# ---- routing configuration (tuned against TimelineSim) ----
DEFAULT_CFG = {
    # heads using Schraudolph fast exp: (u, hh) -> 'pool' | 'dve' (C2 engine)
    "fast": {(1, 1): "pool", (2, 0): "pool", (3, 0): "pool",
             (4, 1): "pool", (5, 0): "pool", (6, 1): "pool"},
    # heads whose PSUM drain runs on ACT (mask-mul then done by DVE in 4x)
    "actdrain": [(0, 1), (2, 1), (4, 0), (6, 0)],
    "qk_eng": "act",
    "v_eng": "act",
    "ctxt_eng": "dve",
    "out_eng": "act",
    "warm": 8,
    "expq": 4,  # true-exp chunk count per head (8/expq * S wide)
}


def _build(bo_nonzero: bool, bv_nonzero: bool = True, cfg=None):
    cfg = dict(DEFAULT_CFG, **(cfg or {}))
    WARM = int(cfg["warm"])
    EXPQ = int(cfg["expq"])
    fast_heads = dict(cfg["fast"])
    actdrain = set(map(tuple, cfg["actdrain"]))
    QK_ENG = cfg["qk_eng"]
    V_ENG = cfg["v_eng"]
    CTXT_ENG = cfg["ctxt_eng"]
    OUT_ENG = cfg["out_eng"]
    assert EXPQ == 3 or 8 % EXPQ == 0, "EXPQ must be 3 or divide 8"

    nc = bacc.Bacc("TRN2", target_bir_lowering=False, debug=False, num_devices=NCORES)

    xT_d = nc.dram_tensor("xT", [BPC, E, S], BF16, kind="ExternalInput").ap()
    maskT_d = nc.dram_tensor("maskT", [BPC, S, S], BF16, kind="ExternalInput").ap()
    wq_d = nc.dram_tensor("wqT", [E, E], BF16, kind="ExternalInput").ap()
    wk_d = nc.dram_tensor("wkT", [E, E], BF16, kind="ExternalInput").ap()
    wv_d = nc.dram_tensor("wvT", [E, E], BF16, kind="ExternalInput").ap()
    wo_d = nc.dram_tensor("woT", [E, E], BF16, kind="ExternalInput").ap()
    bqk_d = nc.dram_tensor("bqk", [128, 8], F32, kind="ExternalInput").ap()
    bv_d = nc.dram_tensor("bv", [E], F32, kind="ExternalInput").ap()
    bo_d = nc.dram_tensor("bo", [E], F32, kind="ExternalInput").ap()
    out_d = nc.dram_tensor("out", [BPC, S, E], F32, kind="ExternalOutput").ap()

    mult = mybir.AluOpType.mult
    add = mybir.AluOpType.add
    EXP = mybir.ActivationFunctionType.Exp
    IDENT = mybir.ActivationFunctionType.Identity
    COPY = mybir.ActivationFunctionType.Copy

    with tile.TileContext(nc) as tc, ExitStack() as ctx:
        singles = ctx.enter_context(tc.tile_pool(name="singles", bufs=1))
        xtp = ctx.enter_context(tc.tile_pool(name="xt", bufs=BPC))
        maskp = ctx.enter_context(tc.tile_pool(name="mask", bufs=BPC))
        qkp = ctx.enter_context(tc.tile_pool(name="qk", bufs=6))
        vp = ctx.enter_context(tc.tile_pool(name="v", bufs=8 * BPC))
        megap = ctx.enter_context(tc.tile_pool(name="mega", bufs=4))
        ctxp = ctx.enter_context(tc.tile_pool(name="ctx", bufs=8 * BPC))
        ctxTp = ctx.enter_context(tc.tile_pool(name="ctxT", bufs=4 * BPC))
        outp = ctx.enter_context(tc.tile_pool(name="outs", bufs=2))
        rcp = ctx.enter_context(tc.tile_pool(name="rc", bufs=8))
        scp = ctx.enter_context(tc.tile_pool(name="sc", bufs=2, space="PSUM"))
        pjp = ctx.enter_context(tc.tile_pool(name="pj", bufs=2, space="PSUM"))
        mmp = ctx.enter_context(tc.tile_pool(name="mm", bufs=2, space="PSUM"))

        # ---- constants ----
        w_sb = {}

        def load_w(name, d, eng=None):
            t = singles.tile([128, 4 * E], BF16, tag=f"w{name}", name=f"w{name}")
            ov = t[:].rearrange("p (c f) -> p c f", c=4)
            iv = d.rearrange("(c p) f -> p c f", p=128)
            (eng or nc.sync).dma_start(out=ov, in_=iv)
            w_sb[name] = t

        bqk_sb = singles.tile([128, 8], F32, tag="bqk")
        nc.sync.dma_start(out=bqk_sb[:], in_=bqk_d[:])
        bv_sb = singles.tile([128, E], F32, tag="bv")
        bo_sb = None
        ident = singles.tile([128, 128], BF16, tag="ident")
        make_identity(nc, ident[:])
        warm_in = singles.tile([128, 512], BF16, tag="warm")
        nc.gpsimd.memset(warm_in[:], 0.0)
        warm_ps = mmp.tile([128, 512], F32, tag="mm", name="warmps")
        for wi in range(WARM):
            nc.tensor.matmul(
                warm_ps[:], lhsT=ident[:], rhs=warm_in[:],
                start=True, stop=True,
            )

        # ---- pipeline state ----
        xt = {}      # b -> x tile
        masks = {}   # b -> mask tile
        qk = {}      # (b, 'q'|'k', j) -> tile [128, S]
        v_sb = {}    # b -> [8 tiles]
        mega = {}    # (u, hh) -> tile (bf16 storage; fast heads bitcast views)
        ctx_sb = {}  # b -> [8 tiles]
        ctxT = {}    # (b, j) -> tile

        def dma_in_x(b, half=None):
            if b in xt:
                t = xt[b]
            else:
                t = xtp.tile([128, 4 * S], BF16, tag="xt", name=f"xt{b}")
                xt[b] = t
            ov = t[:].rearrange("p (e s) -> p e s", e=4)
            iv = xT_d[b].rearrange("(e p) s -> p e s", p=128)
            if half in (None, 0):
                nc.sync.dma_start(out=ov[:, 0:2], in_=iv[:, 0:2])
            if half in (None, 1):
                nc.sync.dma_start(out=ov[:, 2:4], in_=iv[:, 2:4])

        def dma_in_mask(b, pieces=((0, 4), (4, 8))):
            if b in masks:
                t = masks[b]
            else:
                t = maskp.tile([128, 8 * S], BF16, tag="mask", name=f"mask{b}")
                masks[b] = t
            ov = t[:].rearrange("p (sk sq) -> p sk sq", sk=8)
            iv = maskT_d[b].rearrange("(sk p) sq -> p sk sq", p=128)
            for lo, hi in pieces:
                nc.sync.dma_start(out=ov[:, lo:hi], in_=iv[:, lo:hi])

        def dma_in(b):
            dma_in_x(b)
            dma_in_mask(b)

        def drain_qk(t_slice, ps, col):
            if QK_ENG == "act":
                nc.scalar.activation(
                    t_slice, ps[:], IDENT,
                    bias=bqk_sb[:, col : col + 1], scale=1.0,
                )
            else:
                nc.vector.tensor_scalar(
                    t_slice, ps[:], bqk_sb[:, col : col + 1], None, add
                )

        def post_steps(u, hh, mg, b):
            """Deferred per-head post-drain ops: in-place mask multiply (for
            ACT-drained heads), then exp (true) or +C2 (fast). Returned as
            closures so they pipeline into the NEXT head's score loop."""
            fast = (u, hh) in fast_heads
            adrain = (u, hh) in actdrain
            mgw = mg[:].bitcast(FP16) if fast else mg[:]
            steps = []
            if adrain:
                # all-SBUF 16-bit mask multiply: DVE 2x_1p mode (C1 was
                # folded into the ACT drain scale for fast heads)
                def mk_tt(q2):
                    def step():
                        sl = slice(q2 * 4 * S, (q2 + 1) * 4 * S)
                        nc.vector.tensor_tensor(
                            out=mgw[:, sl], in0=mgw[:, sl],
                            in1=masks[b][:, sl], op=mult,
                        )
                    return step

                for q2 in range(2):
                    steps.append(mk_tt(q2))
            if fast:
                # +C2 and round-to-int16: bits become bf16 exp values
                mgi = mg[:].bitcast(I16)
                eng = nc.gpsimd if fast_heads[(u, hh)] == "pool" else nc.vector

                def mk_c2(q2):
                    def step():
                        sl = slice(q2 * 4 * S, (q2 + 1) * 4 * S)
                        eng.tensor_scalar(mgi[:, sl], mgw[:, sl], C2, None, add)
                    return step

                for q2 in range(2):
                    steps.append(mk_c2(q2))
            else:
                def mk_exp(lo, hi):
                    def step():
                        nc.scalar.activation(
                            mg[:, lo * S : hi * S], mg[:, lo * S : hi * S], EXP
                        )
                    return step

                if u == NU - 1:
                    chunks = ((0, 2), (2, 4), (4, 6), (6, 8))
                elif EXPQ == 3:
                    chunks = ((0, 2), (2, 4), (4, 8))
                else:
                    w = 8 // EXPQ
                    chunks = tuple((qq * w, (qq + 1) * w) for qq in range(EXPQ))
                for lo, hi in chunks:
                    steps.append(mk_exp(lo, hi))
            return steps

        def attn_a(u, bsteps, bsteps2=None, carry=None):
            """scores + fused mask-mul for both heads of unit u. Pops backlog
            work (bsteps: PE-side projections/ctx + deferred post-drain ops of
            the previous head) between score tiles so all engines stay fed.
            Returns the deferred post-ops of this unit's second head."""
            b, j = divmod(u, 4)
            kT = qk[(b, "k", j)]
            qT = qk[(b, "q", j)]
            slot = 0
            bsteps = (carry or []) + bsteps
            for hh in range(2):
                if hh == 1 and bsteps2:
                    bsteps = bsteps + bsteps2
                fast = (u, hh) in fast_heads
                adrain = (u, hh) in actdrain
                mg = megap.tile([128, 8 * S], BF16, tag="mega", name=f"mega{u}_{hh}")
                # fast heads carry s*C1*m in fp16 until the +C2 pass
                mgw = mg[:].bitcast(FP16) if fast else mg[:]
                for sk in range(8):
                    ps = scp.tile([128, S], F32, tag="sc", name=f"sc{u}{hh}{sk}")
                    for sh in range(2):
                        nc.tensor.matmul(
                            ps[:, sh * 512 : (sh + 1) * 512],
                            lhsT=kT[hh * 64 : hh * 64 + 64, sk * 128 : sk * 128 + 128],
                            rhs=qT[hh * 64 : hh * 64 + 64, sh * 512 : (sh + 1) * 512],
                            start=True,
                            stop=True,
                        )
                    if adrain:
                        # raw s (scaled by C1 if fast) -> SBUF 16-bit
                        nc.scalar.activation(
                            mgw[:, sk * S : (sk + 1) * S], ps[:], COPY,
                            scale=C1 if fast else 1.0,
                        )
                    else:
                        nc.vector.scalar_tensor_tensor(
                            out=mgw[:, sk * S : (sk + 1) * S],
                            in0=ps[:],
                            scalar=C1 if fast else 1.0,
                            in1=masks[b][:, sk * S : (sk + 1) * S],
                            op0=mult,
                            op1=mult,
                        )
                    slots_left = 16 - slot
                    n = (len(bsteps) + slots_left - 1) // slots_left if bsteps else 0
                    for _ in range(n):
                        if bsteps:
                            bsteps.pop(0)()
                    slot += 1
                mega[(u, hh)] = mg
                posts = post_steps(u, hh, mg, b)
                if hh == 0:
                    # second head's score loop pumps the first head's posts
                    bsteps = posts + bsteps
                else:
                    tail_posts = posts
            for st in bsteps:
                st()
            return tail_posts

        def attn_b_steps(u):
            """ctx matmul + normalize (8 steps) then transposes (4 steps)."""
            b, j = divmod(u, 4)
            steps = []

            def mk_ctx(sq):
                def step():
                    if j == 0 and sq == 0 and b not in ctx_sb:
                        ctx_sb[b] = [
                            ctxp.tile([128, E], BF16, tag="ctx", name=f"ctx{b}_{i}")
                            for i in range(8)
                        ]
                    pc = mmp.tile([128, 130], F32, tag="mm", name=f"pc{u}_{sq}")
                    for hh in range(2):
                        h = 2 * j + hh
                        mg = mega[(u, hh)]
                        for sk in range(8):
                            nc.tensor.matmul(
                                pc[:, hh * 65 : hh * 65 + 65],
                                lhsT=mg[:, sk * S + sq * 128 : sk * S + sq * 128 + 128],
                                rhs=v_sb[b][sk][:, h * 65 : h * 65 + 65],
                                start=(sk == 0),
                                stop=(sk == 7),
                            )
                    rc = rcp.tile([128, 2], F32, tag="rc", name=f"rc{u}_{sq}")
                    pcv = pc[:].rearrange("p (h c) -> p h c", h=2)
                    nc.vector.reciprocal(rc[:], pcv[:, :, 64])
                    if not bv_nonzero:
                        # both heads in one op: in1 = rc broadcast along free
                        in0 = bass.AP(
                            tensor=pc.tensor, offset=pc[:].offset,
                            ap=[pc[:].ap[0], [65, 2], [1, 64]],
                        )
                        in1 = bass.AP(
                            tensor=rc.tensor, offset=rc[:].offset,
                            ap=[rc[:].ap[0], [1, 2], [0, 64]],
                        )
                        osl = ctx_sb[b][sq][:, j * 128 : (j + 1) * 128]
                        out = bass.AP(
                            tensor=osl.tensor, offset=osl.offset,
                            ap=[osl.ap[0], [64, 2], [1, 64]],
                        )
                        nc.vector.tensor_tensor(out=out, in0=in0, in1=in1, op=mult)
                    else:
                        for hh in range(2):
                            h = 2 * j + hh
                            nc.vector.scalar_tensor_tensor(
                                out=ctx_sb[b][sq][:, h * 64 : h * 64 + 64],
                                in0=pc[:, hh * 65 : hh * 65 + 64],
                                scalar=rc[:, hh : hh + 1],
                                in1=bv_sb[:, h * 64 : h * 64 + 64],
                                op0=mult,
                                op1=add,
                            )
                return step

            for sq in range(8):
                steps.append(mk_ctx(sq))

            cT = ctxTp.tile([128, S], BF16, tag="ctxT", name=f"ctxT{b}_{j}")
            ctxT[(b, j)] = cT

            def mk_tr(sq4):
                def step():
                    pt = mmp.tile([128, 512], BF16, tag="mm", name=f"pt{u}_{sq4}")
                    for hh in range(2):
                        for sqi in range(4):
                            sq = sq4 * 4 + sqi
                            nc.tensor.transpose(
                                out=pt[hh * 64 : hh * 64 + 64, sqi * 128 : sqi * 128 + 128],
                                in_=ctx_sb[b][sq][:, (2 * j + hh) * 64 : (2 * j + hh) * 64 + 64],
                                identity=ident[:],
                            )
                    if CTXT_ENG == "dve":
                        nc.vector.tensor_scalar_mul(
                            cT[:, sq4 * 512 : (sq4 + 1) * 512], pt[:], 1.0
                        )
                    else:
                        nc.scalar.activation(
                            cT[:, sq4 * 512 : (sq4 + 1) * 512], pt[:], COPY
                        )
                return step

            for sq4 in range(2):
                steps.append(mk_tr(sq4))
            return steps

        def proj_qk_steps(b, j, split=False):
            steps = []
            for ti, tname in enumerate(("q", "k")):
                t = qkp.tile([128, S], BF16, tag="qk", name=f"qk{b}_{tname}{j}")
                qk[(b, tname, j)] = t
                col = ti * 4 + j

                def mk(tname=tname, t=t, col=col, sh=0):
                    def step():
                        ps = pjp.tile(
                            [128, 512], F32, tag="pj", name=f"pqk{b}{j}{tname}{sh}"
                        )
                        for e in range(4):
                            nc.tensor.matmul(
                                ps[:],
                                lhsT=w_sb[tname][:, e * E + j * 128 : e * E + (j + 1) * 128],
                                rhs=xt[b][:, e * S + sh * 512 : e * S + (sh + 1) * 512],
                                start=(e == 0),
                                stop=(e == 3),
                            )
                        drain_qk(t[:, sh * 512 : (sh + 1) * 512], ps, col)
                    return step

                if split:
                    steps.append(mk(sh=0))
                    steps.append(mk(sh=1))
                else:
                    def mkf(tname=tname, t=t, col=col):
                        def step():
                            ps = scp.tile(
                                [128, S], F32, tag="sc", name=f"pqk{b}{j}{tname}"
                            )
                            for sh in range(2):
                                for e in range(4):
                                    nc.tensor.matmul(
                                        ps[:, sh * 512 : (sh + 1) * 512],
                                        lhsT=w_sb[tname][:, e * E + j * 128 : e * E + (j + 1) * 128],
                                        rhs=xt[b][:, e * S + sh * 512 : e * S + (sh + 1) * 512],
                                        start=(e == 0),
                                        stop=(e == 3),
                                    )
                            drain_qk(t[:], ps, col)
                        return step

                    steps.append(mkf())
            return steps

        def proj_v_steps(b):
            v_sb[b] = [None] * 8
            steps = []

            def mk(s):
                def step():
                    ps = mmp.tile([128, 512], F32, tag="mm", name=f"pv{b}_{s}")
                    for e in range(4):
                        nc.tensor.matmul(
                            ps[:],
                            lhsT=xt[b][:, e * S + s * 128 : e * S + (s + 1) * 128],
                            rhs=w_sb["v"][:, e * E : (e + 1) * E],
                            start=(e == 0),
                            stop=(e == 3),
                        )
                    vt = vp.tile([128, 8 * 65], BF16, tag="v", name=f"v{b}_{s}")
                    vv = vt[:].rearrange("p (h c) -> p h c", h=8)
                    pv = ps[:].rearrange("p (h c) -> p h c", h=8)
                    if V_ENG == "act":
                        nc.scalar.activation(vv[:, :, 0:64], pv[:, :, :], COPY)
                    else:
                        nc.vector.tensor_scalar_mul(vv[:, :, 0:64], pv[:, :, :], 1.0)
                    nc.gpsimd.memset(vv[:, :, 64:65], 1.0)
                    v_sb[b][s] = vt
                return step

            for s in range(8):
                steps.append(mk(s))
            return steps

        def tail_ctx_steps(u, hh):
            """per-head ctx+normalize steps ([128,65] psum tiles) for the last unit."""
            b, j = divmod(u, 4)
            steps = []

            def mk(sq):
                def step():
                    if j == 0 and b not in ctx_sb:
                        ctx_sb[b] = [
                            ctxp.tile([128, E], BF16, tag="ctx", name=f"ctx{b}_{i}")
                            for i in range(8)
                        ]
                    h = 2 * j + hh
                    pc = mmp.tile([128, 65], F32, tag="mm", name=f"tc{u}_{hh}_{sq}")
                    mg = mega[(u, hh)]
                    for sk in range(8):
                        nc.tensor.matmul(
                            pc[:],
                            lhsT=mg[:, sk * S + sq * 128 : sk * S + sq * 128 + 128],
                            rhs=v_sb[b][sk][:, h * 65 : h * 65 + 65],
                            start=(sk == 0),
                            stop=(sk == 7),
                        )
                    rc = rcp.tile([128, 1], F32, tag="rc", name=f"trc{u}_{hh}_{sq}")
                    nc.vector.reciprocal(rc[:], pc[:, 64:65])
                    nc.vector.scalar_tensor_tensor(
                        out=ctx_sb[b][sq][:, h * 64 : h * 64 + 64],
                        in0=pc[:, 0:64],
                        scalar=rc[:, 0:1],
                        in1=bv_sb[:, h * 64 : h * 64 + 64],
                        op0=mult,
                        op1=add,
                    )
                return step

            for sq in range(8):
                steps.append(mk(sq))
            return steps

        def outproj_steps(b, half):
            steps = []
            state = {}

            def mk(si):
                def step():
                    s = half * 4 + si
                    if si % 2 == 0:
                        state["ou"] = outp.tile(
                            [128, 2 * 512], F32, tag="outs", name=f"ou{b}_{s}"
                        )
                    ou = state["ou"]
                    po = mmp.tile([128, 512], F32, tag="mm", name=f"po{b}_{s}")
                    for j in range(4):
                        nc.tensor.matmul(
                            po[:],
                            lhsT=ctxT[(b, j)][:, s * 128 : (s + 1) * 128],
                            rhs=w_sb["o"][:, j * E : (j + 1) * E],
                            start=(j == 0),
                            stop=(j == 3),
                        )
                    osl = ou[:, (si % 2) * 512 : (si % 2 + 1) * 512]
                    if bo_nonzero:
                        nc.vector.scalar_tensor_tensor(
                            out=osl, in0=po[:], scalar=1.0, in1=bo_sb[:],
                            op0=mult, op1=add,
                        )
                    elif OUT_ENG == "act":
                        nc.scalar.activation(osl, po[:], COPY)
                    else:
                        nc.vector.tensor_scalar_mul(osl, po[:], 1.0)
                    if si % 2 == 1:
                        qs = half * 4 + si - 1
                        nc.sync.dma_start(
                            out=out_d[b, qs * 128 : (qs + 1) * 128, :],
                            in_=ou[:, 0:512],
                        )
                        nc.sync.dma_start(
                            out=out_d[b, (qs + 1) * 128 : (qs + 2) * 128, :],
                            in_=ou[:, 512:1024],
                        )
                return step

            for si in range(4):
                steps.append(mk(si))
            return steps

        # ---- pipelined emission ----
        load_w("q", wq_d)
        dma_in_x(0)
        load_w("k", wk_d)
        dma_in_mask(0, pieces=((0, 1), (1, 2), (2, 4), (4, 8)))
        load_w("v", wv_d)
        load_w("o", wo_d)
        nc.sync.dma_start(
            out=bv_sb[:],
            in_=bass.AP(tensor=bv_d.tensor, offset=bv_d.offset, ap=[[0, 128]] + bv_d.ap),
        )
        if bo_nonzero:
            bo_sb = singles.tile([128, E], F32, tag="bo")
            nc.sync.dma_start(
                out=bo_sb[:],
                in_=bass.AP(
                    tensor=bo_d.tensor, offset=bo_d.offset, ap=[[0, 128]] + bo_d.ap
                ),
            )

        for st in proj_qk_steps(0, 0, split=True):
            st()
        carry = None
        for u in range(NU):
            b, j = divmod(u, 4)
            if u == 1 and BPC > 1:
                dma_in(1)
            bsteps = []
            if u + 1 < NU:
                nb, nj = divmod(u + 1, 4)
                bsteps += proj_qk_steps(nb, nj, split=True)
            if j == 0:
                bsteps += proj_v_steps(b)
            if u >= 1:
                bsteps += attn_b_steps(u - 1)
            if u == 5:
                bsteps += outproj_steps(0, 0)
            if u == 6:
                bsteps += outproj_steps(0, 1)
            bsteps2 = tail_ctx_steps(u, 0) if u == NU - 1 else None
            carry = attn_a(u, bsteps, bsteps2, carry=carry)
        for st in carry:  # last head's posts
            st()
        # tail drain: ctx for the second head, interleaved with the last
        # pair's transposes and the final out-projection.
        hh1_steps = tail_ctx_steps(NU - 1, 1)
        b_l, j_l = divmod(NU - 1, 4)
        cT_l = ctxTp.tile([128, S], BF16, tag="ctxT", name=f"ctxT{b_l}_{j_l}")
        ctxT[(b_l, j_l)] = cT_l

        def tr_step(sq4):
            pt = mmp.tile([128, 512], BF16, tag="mm", name=f"ptL_{sq4}")
            for hh in range(2):
                for sqi in range(4):
                    sq = sq4 * 4 + sqi
                    nc.tensor.transpose(
                        out=pt[hh * 64 : hh * 64 + 64, sqi * 128 : sqi * 128 + 128],
                        in_=ctx_sb[b_l][sq][:, (2 * j_l + hh) * 64 : (2 * j_l + hh) * 64 + 64],
                        identity=ident[:],
                    )
            nc.vector.tensor_scalar_mul(
                cT_l[:, sq4 * 512 : (sq4 + 1) * 512], pt[:], 1.0
            )

        for st in hh1_steps:  # hh1 ctx per sq-chunk
            st()
        lb = BPC - 1
        for sq4 in range(2):
            tr_step(sq4)  # transposes + ctxT copy for sq-quad sq4
            for qq in range(2):
                sq2 = sq4 * 2 + qq
                ou = outp.tile([128, 2 * 512], F32, tag="outs", name=f"ouL_{sq2}")
                for si in range(2):
                    s = sq2 * 2 + si
                    po = mmp.tile([128, 512], F32, tag="mm", name=f"poL_{s}")
                    for j in range(4):
                        nc.tensor.matmul(
                            po[:],
                            lhsT=ctxT[(lb, j)][:, s * 128 : (s + 1) * 128],
                            rhs=w_sb["o"][:, j * E : (j + 1) * E],
                            start=(j == 0),
                            stop=(j == 3),
                        )
                    osl = ou[:, si * 512 : (si + 1) * 512]
                    if bo_nonzero:
                        nc.vector.scalar_tensor_tensor(
                            out=osl, in0=po[:], scalar=1.0, in1=bo_sb[:],
                            op0=mult, op1=add,
                        )
                    elif OUT_ENG == "act":
                        nc.scalar.activation(osl, po[:], COPY)
                    else:
                        nc.vector.tensor_scalar_mul(osl, po[:], 1.0)
                nc.sync.dma_start(
                    out=out_d[lb, sq2 * 256 : sq2 * 256 + 128, :], in_=ou[:, 0:512]
                )
                nc.sync.dma_start(
                    out=out_d[lb, sq2 * 256 + 128 : (sq2 + 1) * 256, :],
                    in_=ou[:, 512:1024],
                )

    nc.compile()
    return nc


def _prep(x, adj_matrix, bond_matrix, Wq, bq, Wk, bk, Wv, bv, Wo, bo):
    """Host-side layout prep. Returns per-core input maps."""
    x = np.asarray(x, np.float32)
    mask = np.asarray(adj_matrix, np.float32) + np.asarray(bond_matrix, np.float32)
    xT = np.ascontiguousarray(x.transpose(0, 2, 1)).astype(NPBF16)
    maskT = np.ascontiguousarray(mask.transpose(0, 2, 1)).astype(NPBF16)
    wqT = np.ascontiguousarray(np.asarray(Wq, np.float32).T * SCALE).astype(NPBF16)
    wkT = np.ascontiguousarray(np.asarray(Wk, np.float32).T).astype(NPBF16)
    wvT = np.ascontiguousarray(np.asarray(Wv, np.float32).T).astype(NPBF16)
    woT = np.ascontiguousarray(np.asarray(Wo, np.float32).T).astype(NPBF16)
    bqs = np.asarray(bq, np.float32) * SCALE
    bkf = np.asarray(bk, np.float32)
    # [128, 8]: cols 0-3 = bq chunks, 4-7 = bk chunks (chunk c = f in [128c,128c+128))
    bqk = np.concatenate(
        [bqs.reshape(4, 128).T, bkf.reshape(4, 128).T], axis=1
    ).astype(np.float32)
    bqk = np.ascontiguousarray(bqk)
    bvf = np.ascontiguousarray(np.asarray(bv, np.float32))
    bof = np.ascontiguousarray(np.asarray(bo, np.float32))

    in_maps = []
    for c in range(NCORES):
        sl = slice(c * BPC, (c + 1) * BPC)
        in_maps.append(
            {
                "xT": np.ascontiguousarray(xT[sl]),
                "maskT": np.ascontiguousarray(maskT[sl]),
                "wqT": wqT,
                "wkT": wkT,
                "wvT": wvT,
                "woT": woT,
                "bqk": bqk,
                "bv": bvf,
                "bo": bof,
            }
        )
    return in_maps, bool(np.any(bof))


def kernel(
    x,
    adj_matrix,
    bond_matrix,
    Wq,
    bq,
    Wk,
    bk,
    Wv,
    bv,
    Wo,
    bo,
    seq_len,
    _trace=False,
):
    in_maps, bo_nonzero = _prep(
        x, adj_matrix, bond_matrix, Wq, bq, Wk, bk, Wv, bv, Wo, bo
    )
    bv_nonzero = bool(np.any(np.asarray(bv)))
    key = ("k", bo_nonzero, bv_nonzero)
    if key not in _cache:
        _cache[key] = _build(bo_nonzero, bv_nonzero)
    nc = _cache[key]
    res = run_bass_kernel_spmd(
        nc, in_maps, core_ids=list(range(NCORES)), trace=_trace
    )
    out = np.concatenate([r["out"] for r in res.results], axis=0).astype(np.float32)
    if _trace:
        kernel._last_exec_time_ns = res.exec_time_ns
        kernel._last_results = res
    return out
